# revision 33
# baseline (speedup 1.0000x reference)
"""Trainium2 Bass kernel: LSTM url encoder + 2-layer hetero GCN + edge classifier,
sharded over 8 NeuronCores (nodes re-labeled to per-core slots with identical
length profile; message passing via dma_gather + 0/1-indicator PSUM matmuls;
feature tables replicated with AllGather; BatchNorm stats via AllReduce)."""
import sys, types
import numpy as np

try:
    from trn_agent_boot.trn_boot import _ntff_profile_via_ctypes
    _m = types.ModuleType("antenv.axon_hooks")
    _m.get_axon_ntff_profile_hook = lambda: _ntff_profile_via_ctypes('/opt/axon/libaxon_pjrt.so')
    _m.set_axon_ntff_profile_hook = lambda h: None
    sys.modules.setdefault("antenv.axon_hooks", _m)
except Exception:
    pass

import concourse.bass as bass
import concourse.tile as tile
from concourse import bacc, mybir
from concourse.bass_utils import run_bass_kernel_spmd
from concourse.library_config import mlp
from concourse.masks import make_identity
import ml_dtypes

F32 = mybir.dt.float32
BF16 = mybir.dt.bfloat16
NPBF = ml_dtypes.bfloat16
I8 = mybir.dt.int8
I16 = mybir.dt.int16
ALU = mybir.AluOpType

NC = 8
N, L, E, H = 50000, 32, 64, 128
NSLOT = 6656
NBLK, BW = 13, 512
NTILE = NSLOT // 128
NGLOB = NC * NSLOT
ZROW = NGLOB
PAIRZ = NGLOB // 2     # zeroed pad pair-row in the p1/p2 tables
ESUB = 200000
ESH = ESUB // NC
GRP = 4                # dst tiles per merged gather group


def _wrap_idx(iv):
    n = len(iv)
    w = np.zeros((128, n // 16), np.int16)
    blk = iv.reshape(n // 16, 16).T
    for g in range(8):
        w[16 * g:16 * (g + 1)] = blk
    return w


def host_prep(inp):
    lens = np.asarray(inp["inputs_sm"]).astype(np.int64).sum(1)
    ids = np.asarray(inp["inputs_s"]).astype(np.int64)
    order = np.argsort(lens, kind="stable")
    g_of = np.zeros(N, np.int64)
    slot_node = np.full((NC, NSLOT), -1, np.int64)
    lens_sorted = lens[order]
    prof = []
    per_core_pos = 0
    idx0 = 0
    while idx0 < N:
        l = int(lens_sorted[idx0])
        idx1 = idx0
        while idx1 < N and lens_sorted[idx1] == l:
            idx1 += 1
        nodes = order[idx0:idx1]
        m = -(-(idx1 - idx0) // NC)
        for j, nd in enumerate(nodes):
            c, p = j % NC, per_core_pos + j // NC
            slot_node[c, p] = nd
            g_of[nd] = c * NSLOT + p
        prof.extend([l] * m)
        per_core_pos += m
        idx0 = idx1
    prof.extend([prof[-1]] * (NSLOT - per_core_pos))
    slot_len = np.array(prof, np.int64)
    steps_chunk = [int(slot_len[min(k * 128 + 127, NSLOT - 1)]) for k in range(NTILE)]
    steps_blk = [max(steps_chunk[b * 4:(b + 1) * 4]) for b in range(NBLK)]
    harvests = []
    for b in range(NBLK):
        ev = []
        ls = slot_len[b * BW:(b + 1) * BW]
        for l in np.unique(ls):
            w = np.where(ls == l)[0]
            ev.append((int(l) - 1, int(w[0]), int(w[-1]) + 1))
        harvests.append(ev)
    TOTS = sum(steps_blk)
    meta = dict(steps_blk=steps_blk, harvests=harvests, TOTS=TOTS)

    f32 = lambda a: np.ascontiguousarray(a, np.float32)
    bf16 = lambda a: np.ascontiguousarray(np.asarray(a, np.float32), NPBF)
    # gate column order [i, f, o, g] (PyTorch i,f,g,o reordered)
    GPERM = np.r_[0:E, E:2 * E, 3 * E:4 * E, 2 * E:3 * E]
    emb_url = f32(inp["emb_url"])
    shared = {}
    GSC = np.ones(4 * E, np.float32)
    GSC[:3 * E] = 0.5          # tanh(0.5*x) for i,f,o; tanh(x) for g
    for suf in ("f", "b"):
        ep = emb_url @ f32(inp["Wih_" + suf]).T + f32(inp["b_" + suf])[None, :]
        shared[f"embproj_{suf}"] = bf16(ep[:, GPERM] * GSC[None, :])
    whhp = np.zeros((128, 4 * E), np.float32)
    whhp[0:E] = 0.5 * f32(inp["Whh_f"]).T[:, GPERM] * GSC[None, :]
    whhp[E:128] = 0.5 * f32(inp["Whh_b"]).T[:, GPERM] * GSC[None, :]
    shared["whhT"] = bf16(whhp)
    shared["fcWT"] = bf16(0.5 * f32(inp["fc_W"]).T)
    fcb = np.zeros((128, 1), np.float32)
    fcb[0:E, 0] = f32(inp["fc_b"])
    shared["fcb"] = fcb
    slope = np.ones((128, 1), np.float32)
    slope[0:E, 0] = 0.01
    shared["slope_a"] = slope
    for r in ("sim", "user"):
        shared[f"W0{r}a"] = bf16(f32(inp[f"gcn0_{r}_W"])[:128])
        shared[f"W0{r}b"] = bf16(f32(inp[f"gcn0_{r}_W"])[128:])
        shared[f"W1{r}"] = bf16(f32(inp[f"gcn1_{r}_W"]))
    # cls_b1 is dropped: BatchNorm with batch stats cancels any pre-BN bias.
    shared["W1aT"] = f32(inp["cls_W1"])[:, :H].T.copy()
    shared["W1bT"] = f32(inp["cls_W1"])[:, H:].T.copy()
    shared["bngb"] = np.stack([f32(inp["bn_g"]), f32(inp["bn_b"])], 1)
    shared["W2T"] = bf16(f32(inp["cls_W2"]).T)
    shared["b2bias"] = np.tile(f32(inp["cls_b2"])[None, :], (128, 1))
    for lyr in (0, 1):
        bb = f32(inp[f"gcn{lyr}_sim_b"]) + f32(inp[f"gcn{lyr}_user_b"])
        shared[f"biasL{lyr}"] = np.tile(bb[None, :], (128, 1))

    edges, deg = {}, {}
    for r in ("sim", "user"):
        src = g_of[np.asarray(inp[r + "_src"]).astype(np.int64)]
        dst = g_of[np.asarray(inp[r + "_dst"]).astype(np.int64)]
        od = np.zeros(NGLOB, np.float32); np.add.at(od, src, 1.0)
        idg = np.zeros(NGLOB, np.float32); np.add.at(idg, dst, 1.0)
        deg[r] = (np.maximum(od, 1.0), np.maximum(idg, 1.0))
        edges[r] = (src, dst)

    # GCN edges: chunks of 128 edges per dst tile, split by src parity so a
    # single bf16 pair-table [NGLOB//2, 2H] serves int16 indices (idx = src>>1).
    groups = [list(range(g, min(g + GRP, NTILE))) for g in range(0, NTILE, GRP)]
    gcn = {}
    for r in ("sim", "user"):
        src, dst = edges[r]
        core = dst // NSLOT
        dl = dst % NSLOT
        tt = dl // 128
        by = [[[[], []] for _ in range(NTILE)] for _ in range(NC)]
        for c in range(NC):
            m = np.where(core == c)[0]
            for e in m:
                by[c][tt[e]][int(src[e]) & 1].append((int(src[e]), int(dl[e]) % 128))
        KE = [max(1, max(-(-len(by[c][t][0]) // 128) for c in range(NC))) for t in range(NTILE)]
        KO = [max(1, max(-(-len(by[c][t][1]) // 128) for c in range(NC))) for t in range(NTILE)]
        gcn[r] = dict(KE=KE, KO=KO, by=by)
    meta["gcn"] = {r: dict(KE=gcn[r]["KE"], KO=gcn[r]["KO"]) for r in ("sim", "user")}
    meta["groups"] = groups

    es = g_of[np.asarray(inp["esub_src"]).astype(np.int64)]
    ed = g_of[np.asarray(inp["esub_dst"]).astype(np.int64)]
    cls_bk = []
    for c in range(NC):
        sl = slice(c * ESH, (c + 1) * ESH)
        s_c, d_c, gi = es[sl], ed[sl], np.arange(c * ESH, (c + 1) * ESH)
        bks = []
        for bi in range(4):
            m = ((s_c & 1) * 2 + (d_c & 1)) == bi
            bks.append((s_c[m], d_c[m], gi[m]))
        cls_bk.append(bks)
    CBK = [max(1, max(-(-len(cls_bk[c][bi][0]) // 128) for c in range(NC))) for bi in range(4)]
    CC = sum(CBK)
    meta.update(CC=CC, CBK=CBK)

    in_maps, out_rows = [], []
    for c in range(NC):
        m = dict(shared)
        ohc = np.zeros((2, TOTS, 128, BW), NPBF)
        off = 0
        for b in range(NBLK):
            nd = slot_node[c, b * BW:(b + 1) * BW]
            ln = slot_len[b * BW:(b + 1) * BW]
            real = nd >= 0
            idsF = np.zeros((BW, L), np.int64)
            idsB = np.zeros((BW, L), np.int64)
            idsF[real] = ids[nd[real]]
            for j in np.where(real)[0]:
                lj = int(lens[nd[j]])
                idsB[j, :lj] = ids[nd[j], :lj][::-1]
            for s in range(steps_blk[b]):
                w = np.where((s < ln) & real)[0]
                ohc[0, off, idsF[w, s], w] = 1
                ohc[1, off, idsB[w, s], w] = 1
                off += 1
        m["oh_f"] = np.ascontiguousarray(ohc[0].reshape(TOTS * 128, BW))
        m["oh_b"] = np.ascontiguousarray(ohc[1].reshape(TOTS * 128, BW))
        for name, key in (("ohcat", "inputs_c"), ("ohcou", "inputs_co"), ("ohsl", "inputs_sl")):
            o = np.zeros((128, NSLOT), NPBF)
            nd = slot_node[c]
            real = nd >= 0
            vals = np.zeros(NSLOT, np.int64)
            vals[real] = np.asarray(inp[key]).astype(np.int64)[nd[real], 0]
            o[vals, np.arange(NSLOT)] = 1
            m[name] = o
        m["ecat"] = np.zeros((128, E), NPBF); m["ecat"][:101] = bf16(inp["emb_cat"])
        m["ecou"] = np.zeros((128, E), NPBF); m["ecou"][:92] = bf16(inp["emb_country"])
        m["esl"] = np.zeros((128, E), NPBF); m["esl"][:6] = bf16(inp["emb_sl"])
        sl = slice(c * NSLOT, (c + 1) * NSLOT)
        for r in ("sim", "user"):
            od, idg = deg[r]
            m[f"odeg_{r}"] = np.ascontiguousarray(od[sl].reshape(NTILE, 128).T)
            m[f"ideg_{r}"] = np.ascontiguousarray(idg[sl].reshape(NTILE, 128).T)
        for r in ("sim", "user"):
            KE, KO, by = gcn[r]["KE"], gcn[r]["KO"], gcn[r]["by"]
            idx_all, ind_cols = [], []
            for grp in groups:
                for parity, KX in ((0, KE), (1, KO)):
                    for t in grp:
                        KK = KX[t]
                        lst = by[c][t][parity]
                        arr = np.zeros((KK * 128, 2), np.int64)
                        ok = np.zeros(KK * 128, bool)
                        if lst:
                            a = np.array(lst, np.int64)
                            arr[:len(a)] = a
                            ok[:len(a)] = True
                        iv = (arr[:, 0] >> 1).astype(np.int16)
                        iv[~ok] = 0
                        idx_all.append(iv)
                        mm = np.zeros((KK * 128, 128), np.int8)
                        mm[np.where(ok)[0], arr[ok, 1]] = 1
                        ind_cols.append(mm.reshape(KK, 128, 128).transpose(1, 0, 2).reshape(128, KK * 128))
            m[f"gidx_{r}"] = _wrap_idx(np.concatenate(idx_all))
            m[f"gind_{r}"] = np.ascontiguousarray(np.concatenate(ind_cols, 1).astype(NPBF))
        rows = np.full(CC * 128, -1, np.int64)
        sidx = np.zeros(CC * 128, np.int64); didx = np.zeros(CC * 128, np.int64)
        o = 0
        for bi in range(4):
            s_c, d_c, gi = cls_bk[c][bi]
            nn_ = len(s_c)
            sidx[o:o + nn_] = s_c; didx[o:o + nn_] = d_c; rows[o:o + nn_] = gi
            sidx[o + nn_:o + CBK[bi] * 128] = 2 * PAIRZ; didx[o + nn_:o + CBK[bi] * 128] = 2 * PAIRZ
            o += CBK[bi] * 128
        m["cs_idx"] = _wrap_idx((sidx >> 1).astype(np.int16))
        m["cd_idx"] = _wrap_idx((didx >> 1).astype(np.int16))
        out_rows.append(rows)
        in_maps.append(m)
    return in_maps, out_rows, meta


def build_program(meta):
    nc = bacc.Bacc("TRN2", target_bir_lowering=False, debug=False, num_devices=NC,
                   num_swdge_queues=4)
    TOTS, CC = meta["TOTS"], meta["CC"]
    ein = lambda n, s, d=F32: nc.dram_tensor(n, s, d, kind="ExternalInput")
    x = {}
    for d in ("f", "b"):
        x["embproj_" + d] = ein("embproj_" + d, [128, 4 * E], BF16)
        x["oh_" + d] = ein("oh_" + d, [TOTS * 128, BW], BF16)
    for nm, sh, dt in (("whhT", [128, 4 * E], BF16),
                       ("fcWT", [2 * E, E], BF16), ("fcb", [128, 1], F32),
                       ("slope_a", [128, 1], F32),
                       ("ohcat", [128, NSLOT], BF16), ("ohcou", [128, NSLOT], BF16),
                       ("ohsl", [128, NSLOT], BF16), ("ecat", [128, E], BF16),
                       ("ecou", [128, E], BF16), ("esl", [128, E], BF16),
                       ("W1aT", [H, H], F32), ("W1bT", [H, H], F32),
                       ("W2T", [H, 2], BF16),
                       ("bngb", [128, 2], F32),
                       ("b2bias", [128, 2], F32), ("biasL0", [128, H], F32),
                       ("biasL1", [128, H], F32),
                       ("cs_idx", [128, CC * 8], I16), ("cd_idx", [128, CC * 8], I16)):
        x[nm] = ein(nm, sh, dt)
    for r in ("sim", "user"):
        x[f"W0{r}a"] = ein(f"W0{r}a", [H, H], BF16)
        x[f"W0{r}b"] = ein(f"W0{r}b", [H, H], BF16)
        x[f"W1{r}"] = ein(f"W1{r}", [H, H], BF16)
    for r in ("sim", "user"):
        TK = sum(meta["gcn"][r]["KE"]) + sum(meta["gcn"][r]["KO"])
        x[f"gidx_{r}"] = ein(f"gidx_{r}", [128, TK * 8], I16)
        x[f"gind_{r}"] = ein(f"gind_{r}", [128, TK * 128], BF16)
        x[f"odeg_{r}"] = ein(f"odeg_{r}", [128, NTILE])
        x[f"ideg_{r}"] = ein(f"ideg_{r}", [128, NTILE])
    out = nc.dram_tensor("out", [128, CC * 2], F32, kind="ExternalOutput")

    feat_in = {k: nc.dram_tensor(f"fin_{k}", [NSLOT, H], BF16) for k in ("0sim", "0user", "1sim", "1user")}
    feat_pair = {k: nc.dram_tensor(f"fp_{k}", [NGLOB // 2, 2 * H], BF16, addr_space="Shared")
                 for k in ("0sim", "0user", "1sim", "1user")}
    p_in = {k: nc.dram_tensor(f"pin_{k}", [NSLOT, H], F32) for k in ("p1", "p2")}
    p_pair = {k: nc.dram_tensor(f"pp_{k}", [PAIRZ + 64, 2 * H], F32, addr_space="Shared")
              for k in ("p1", "p2")}
    zt_dram = nc.dram_tensor("zt_dram", [128, CC * 128], F32)
    st_in = nc.dram_tensor("st_in", [128, 2], F32)
    st_out = nc.dram_tensor("st_out", [128, 2], F32, addr_space="Shared")
    RG = [list(range(NC))]
    A = mybir.ActivationFunctionType
    groups = meta["groups"]
    import os
    DBG = bool(os.environ.get("KDEBUG"))
    dbg = {}
    if DBG:
        dbg["f0sim"] = nc.dram_tensor("dbg_f0sim", [NGLOB // 2, 2 * H], BF16, kind="ExternalOutput")
        dbg["h1T"] = nc.dram_tensor("dbg_h1T", [128, NSLOT], BF16, kind="ExternalOutput")
        dbg["p1"] = nc.dram_tensor("dbg_p1", [PAIRZ + 64, 2 * H], F32, kind="ExternalOutput")
        dbg["p2"] = nc.dram_tensor("dbg_p2", [PAIRZ + 64, 2 * H], F32, kind="ExternalOutput")
        dbg["st"] = nc.dram_tensor("dbg_st", [128, 2], F32, kind="ExternalOutput")

    with tile.TileContext(nc) as tc:
        nc.gpsimd.load_library(mlp)
        with tc.tile_pool(name="const", bufs=1) as cpool, \
             tc.tile_pool(name="stage", bufs=1) as spool:
            ident = cpool.tile([128, 128], F32)
            make_identity(nc, ident)
            identb = cpool.tile([128, 128], BF16, tag="identb")
            nc.vector.tensor_copy(identb[:], ident[:])
            onecol = cpool.tile([128, 1], F32, tag="onecol")
            nc.vector.memset(onecol[:], 1.0)
            hsp_cm = tc.tile_pool(name="hstage", bufs=1)
            hsp = hsp_cm.__enter__()
            LSTM_ONLY = {"embproj_f", "embproj_b", "whhT", "fcWT", "fcb",
                         "slope_a", "ecat", "ecou", "esl", "W0sima", "W0simb",
                         "W0usera", "W0userb"}
            consts = {}
            for nm in ("embproj_f", "embproj_b", "whhT",
                       "fcWT", "fcb", "slope_a", "ecat", "ecou", "esl",
                       "W0sima", "W0simb", "W0usera", "W0userb",
                       "W1sim", "W1user", "W1aT", "W1bT", "W2T",
                       "bngb", "b2bias", "biasL0", "biasL1"):
                pool_ = hsp if nm in LSTM_ONLY else cpool
                t = pool_.tile(list(x[nm].shape), x[nm].dtype, tag=nm)
                nc.sync.dma_start(t[:], x[nm][:])
                consts[nm] = t
            idxt = {}
            for nm in ("gidx_sim", "gidx_user", "cs_idx", "cd_idx"):
                t = cpool.tile(list(x[nm].shape), I16, tag="i" + nm)
                nc.sync.dma_start(t[:], x[nm][:])
                idxt[nm] = t
            norm = {}
            for r in ("sim", "user"):
                for kind in ("odeg", "ideg"):
                    t = cpool.tile([128, NTILE], F32, tag=f"d{kind}{r}")
                    nc.sync.dma_start(t[:], x[f"{kind}_{r}"][:])
                    s = cpool.tile([128, NTILE], F32, tag=f"s{kind}{r}")
                    nc.scalar.sqrt(s[:], t[:])
                    rv = cpool.tile([128, NTILE], F32, tag=f"r{kind}{r}")
                    nc.vector.reciprocal(rv[:], s[:])
                    norm[f"{kind}_{r}"] = rv

            hhT = hsp.tile([128, NSLOT], BF16, tag="bigh")
            # ---------- LSTM ----------
            # Packed layout: partitions 0:64 = forward dir, 64:128 = backward.
            # h' = 2h (fold into whhT/fcWT at host); sigmoid via tanh:
            # sigma(x) = 0.5*tanh(0.5x) + 0.5. Gate bias folded into embproj.
            off_blk = [0]
            for b in range(NBLK):
                off_blk.append(off_blk[-1] + meta["steps_blk"][b])
            with tc.tile_pool(name="lwork", bufs=3) as wp, \
                 tc.tile_pool(name="lstate", bufs=2) as lsp, \
                 tc.tile_pool(name="lps", bufs=1, space="PSUM") as pp:
                def lstm_step(b, s, par, S_b, h_b):
                    base = (off_blk[b] + s) * 128
                    ohf = wp.tile([128, BW], BF16, tag=f"ohf{par}")
                    nc.sync.dma_start(ohf[:], x["oh_f"][base:base + 128, :])
                    ohb = wp.tile([128, BW], BF16, tag=f"ohb{par}")
                    nc.sync.dma_start(ohb[:], x["oh_b"][base:base + 128, :])
                    ps3 = pp.tile([128, 3 * BW], F32, tag=f"ifo{par}")
                    psg = pp.tile([128, BW], F32, tag=f"gg{par}")
                    outs = [ps3[:, 0:BW], ps3[:, BW:2 * BW], ps3[:, 2 * BW:3 * BW], psg[:]]
                    for j in range(4):
                        g64 = slice(j * E, (j + 1) * E)
                        o = outs[j]
                        nc.tensor.matmul(o[0:E, :], consts["embproj_f"][:, g64], ohf[:], start=True, stop=False)
                        nc.tensor.matmul(o[E:128, :], consts["embproj_b"][:, g64], ohb[:], start=True, stop=False)
                    for j in range(4):
                        g64 = slice(j * E, (j + 1) * E)
                        o = outs[j]
                        nc.tensor.matmul(o[0:E, :], consts["whhT"][0:E, g64], h_b[0:E, :], start=False, stop=True)
                        nc.tensor.matmul(o[E:128, :], consts["whhT"][E:128, g64], h_b[E:128, :], start=False, stop=True)
                    tifo = wp.tile([128, 3 * BW], BF16, tag=f"tifo{par}")
                    nc.scalar.activation(tifo[:], ps3[:], A.Tanh)
                    tg = wp.tile([128, BW], BF16, tag=f"tg{par}")
                    nc.scalar.activation(tg[:], psg[:], A.Tanh)
                    A2 = wp.tile([128, BW], F32, tag=f"A2{par}")
                    nc.vector.scalar_tensor_tensor(A2[:], tifo[:, BW:2 * BW], 1.0, S_b[:], ALU.add, ALU.mult)
                    B2 = wp.tile([128, BW], BF16, tag=f"B2{par}")
                    nc.vector.scalar_tensor_tensor(B2[:], tifo[:, 0:BW], 1.0, tg[:], ALU.add, ALU.mult)
                    nc.vector.scalar_tensor_tensor(S_b[:], A2[:], 0.5, B2[:], ALU.mult, ALU.add)
                    tc_ = wp.tile([128, BW], BF16, tag=f"tc{par}")
                    nc.scalar.activation(tc_[:], S_b[:], A.Tanh, scale=0.5)
                    nc.vector.scalar_tensor_tensor(h_b[:], tifo[:, 2 * BW:3 * BW], 1.0, tc_[:], ALU.add, ALU.mult)
                    for (sd, c0, c1) in meta["harvests"][b]:
                        if sd == s:
                            nc.vector.tensor_copy(hhT[:, b * BW + c0:b * BW + c1], h_b[:, c0:c1])

                bpairs = [tuple(range(b, min(b + 2, NBLK))) for b in range(0, NBLK, 2)]
                for pair in bpairs:
                    st = {}
                    for par, b in enumerate(pair):
                        S_b = lsp.tile([128, BW], F32, tag=f"S{par}")
                        h_b = lsp.tile([128, BW], BF16, tag=f"h{par}")
                        nc.vector.memset(S_b[:], 0.0)
                        nc.vector.memset(h_b[:], 0.0)
                        st[b] = (S_b, h_b)
                    smax = max(meta["steps_blk"][b] for b in pair)
                    for s in range(smax):
                        for par, b in enumerate(pair):
                            if s < meta["steps_blk"][b]:
                                lstm_step(b, s, par, *st[b])

            # ---------- fc + embeds + feat0 proj ----------
            h0a = hsp.tile([128, NSLOT], BF16, tag="big2")
            h0b = hsp.tile([128, NSLOT], BF16, tag="big3")
            with tc.tile_pool(name="fwork", bufs=3) as wp, \
                 tc.tile_pool(name="fps", bufs=2, space="PSUM") as pp:
                for b in range(NBLK):
                    sl_ = slice(b * BW, (b + 1) * BW)
                    psa = pp.tile([128, BW], F32, tag="psa")
                    psb = pp.tile([128, BW], F32, tag="psb")
                    nc.tensor.matmul(psa[0:E, :], consts["fcWT"][:], hhT[:, sl_], start=True, stop=True)
                    for (ohn, etab, ps_, dr) in (("ohcat", "ecat", psa, slice(E, 128)),
                                                 ("ohcou", "ecou", psb, slice(0, E)),
                                                 ("ohsl", "esl", psb, slice(E, 128))):
                        ohp = wp.tile([128, BW], BF16, tag="oh" + ohn[2:])
                        nc.sync.dma_start(ohp[:], x[ohn][:, sl_])
                        nc.tensor.matmul(ps_[dr, :], consts[etab][:], ohp[:], start=True, stop=True)
                    za = wp.tile([128, BW], F32, tag="za")
                    nc.scalar.activation(za[:], psa[:], A.Identity, bias=consts["fcb"][:, :])
                    nc.vector.scalar_tensor_tensor(h0a[:, sl_], za[:], consts["slope_a"][:, 0:1], za[:], ALU.mult, ALU.max)
                    nc.vector.tensor_copy(h0b[:, sl_], psb[:])
                for r in ("sim", "user"):
                    for k in range(NTILE):
                        sl_ = slice(k * 128, (k + 1) * 128)
                        ps = pp.tile([128, H], F32, tag="proj")
                        nc.tensor.matmul(ps[:], h0a[:, sl_], consts[f"W0{r}a"][:], start=True, stop=False)
                        nc.tensor.matmul(ps[:], h0b[:, sl_], consts[f"W0{r}b"][:], start=False, stop=True)
                        ot = wp.tile([128, H], BF16, tag="po")
                        nc.vector.tensor_scalar_mul(ot[:], ps[:], norm[f"odeg_{r}"][:, k:k + 1])
                        nc.sync.dma_start(feat_in["0" + r][sl_, :], ot[:])
                    nc.gpsimd.collective_compute("AllGather", ALU.bypass, replica_groups=RG,
                                                 ins=[feat_in["0" + r][:]], outs=[feat_pair["0" + r][:]])
            hsp_cm.__exit__(None, None, None)

            # ---------- GCN ----------
            # Gathers use pair-tables [NGLOB//2, 2H]: idx = src >> 1 (int16-safe),
            # elem_step = 2 rows, base column offset selects src parity. Chunks are
            # parity-homogeneous; gather calls merged over GRP dst tiles, spread
            # over 4 SWDGE queues (parallel Q7 desc-gen on 4 core pairs).
            def gcn_layer(lyr, h_next_T):
                with tc.tile_pool(name=f"gw{lyr}", bufs=2) as wp, \
                     tc.tile_pool(name=f"gg{lyr}", bufs=2) as gp, \
                     tc.tile_pool(name=f"gp{lyr}", bufs=2, space="PSUM") as pp, \
                     tc.tile_pool(name=f"gt{lyr}", bufs=1, space="PSUM") as pt:
                    idx_off = {r: 0 for r in ("sim", "user")}
                    ind_off = {r: 0 for r in ("sim", "user")}
                    for gi, grp in enumerate(groups):
                        pools = {}
                        for qb, r in ((0, "sim"), (2, "user")):
                            KE = [meta["gcn"][r]["KE"][t] for t in grp]
                            KO = [meta["gcn"][r]["KO"][t] for t in grp]
                            KEg, KOg = sum(KE), sum(KO)
                            pe = gp.tile([128, KEg, H], BF16, tag=f"pe{r}")
                            po = gp.tile([128, KOg, H], BF16, tag=f"po{r}")
                            io = idx_off[r]
                            nc.gpsimd.dma_gather(pe[:], feat_pair[f"{lyr}{r}"][:, 0:H],
                                                 idxt[f"gidx_{r}"][:, io // 16:(io + KEg * 128) // 16],
                                                 KEg * 128, KEg * 128, H, elem_step=2 * H,
                                                 single_packet=False, queue_num=qb)
                            nc.gpsimd.dma_gather(po[:], feat_pair[f"{lyr}{r}"][:, H:2 * H],
                                                 idxt[f"gidx_{r}"][:, (io + KEg * 128) // 16:(io + (KEg + KOg) * 128) // 16],
                                                 KOg * 128, KOg * 128, H, elem_step=2 * H,
                                                 single_packet=False, queue_num=qb + 1)
                            idx_off[r] = io + (KEg + KOg) * 128
                            ind = gp.tile([128, (KEg + KOg) * 128], BF16, tag=f"ind{r}")
                            nc.sync.dma_start(ind[:], x[f"gind_{r}"][:, ind_off[r]:ind_off[r] + (KEg + KOg) * 128])
                            ind_off[r] += (KEg + KOg) * 128
                            pools[r] = (pe, po, ind, KE, KO, KEg)
                        for ti, t in enumerate(grp):
                            res = {}
                            for r in ("sim", "user"):
                                pe, po, ind, KE, KO, KEg = pools[r]
                                eb, ob = sum(KE[:ti]), sum(KO[:ti])
                                ps = pp.tile([128, H], F32, tag="sc" + r)
                                K = KE[ti] + KO[ti]
                                kk = 0
                                for j in range(KE[ti]):
                                    nc.tensor.matmul(ps[:], ind[:, (eb + j) * 128:(eb + j + 1) * 128],
                                                     pe[:, eb + j, :], start=(kk == 0), stop=(kk == K - 1))
                                    kk += 1
                                for j in range(KO[ti]):
                                    nc.tensor.matmul(ps[:], ind[:, (KEg + ob + j) * 128:(KEg + ob + j + 1) * 128],
                                                     po[:, ob + j, :], start=(kk == 0), stop=(kk == K - 1))
                                    kk += 1
                                res[r] = ps
                            t1 = wp.tile([128, H], F32, tag="e1")
                            nc.vector.tensor_scalar_mul(t1[:], res["sim"][:], norm["ideg_sim"][:, t:t + 1])
                            t2 = wp.tile([128, H], F32, tag="e2")
                            nc.vector.scalar_tensor_tensor(t2[:], res["user"][:], norm["ideg_user"][:, t:t + 1],
                                                           t1[:], ALU.mult, ALU.add)
                            t3 = wp.tile([128, H], F32, tag="e3")
                            nc.vector.tensor_add(t3[:], t2[:], consts[f"biasL{lyr}"][:])
                            if h_next_T is not None:
                                hrow = wp.tile([128, H], BF16, tag="e4")
                                nc.vector.scalar_tensor_tensor(hrow[:], t3[:], 0.01, t3[:], ALU.mult, ALU.max)
                                ps2 = pt.tile([128, H], BF16, tag="tr")
                                nc.tensor.transpose(ps2[:], hrow[:], identb[:])
                                nc.vector.tensor_copy(h_next_T[:, t * 128:(t + 1) * 128], ps2[:])
                            else:
                                hrow = wp.tile([128, H], F32, tag="e4f")
                                nc.vector.scalar_tensor_tensor(hrow[:], t3[:], 0.01, t3[:], ALU.mult, ALU.max)
                                ps2 = pt.tile([128, H], F32, tag="trf")
                                nc.tensor.transpose(ps2[:], hrow[:], ident[:])
                                h2T = wp.tile([128, H], F32, tag="h2T")
                                nc.vector.tensor_copy(h2T[:], ps2[:])
                                for pk, wk in (("p1", "W1aT"), ("p2", "W1bT")):
                                    psp = pt.tile([128, H], F32, tag="pp" + pk)
                                    nc.tensor.matmul(psp[:], h2T[:], consts[wk][:], start=True, stop=True)
                                    pr = wp.tile([128, H], F32, tag="pr" + pk)
                                    nc.vector.tensor_copy(pr[:], psp[:])
                                    nc.sync.dma_start(p_in[pk][t * 128:(t + 1) * 128, :], pr[:])

            h1T = spool.tile([128, NSLOT], BF16, tag="big1")
            gcn_layer(0, h1T)
            if DBG:
                nc.sync.dma_start(dbg["h1T"][:], h1T[:])
            with tc.tile_pool(name="pw", bufs=2) as wp, \
                 tc.tile_pool(name="pp1", bufs=4, space="PSUM") as pp:
                for r in ("sim", "user"):
                    for k in range(NTILE):
                        sl_ = slice(k * 128, (k + 1) * 128)
                        ps = pp.tile([128, H], F32, tag="proj")
                        nc.tensor.matmul(ps[:], h1T[:, sl_], consts[f"W1{r}"][:], start=True, stop=True)
                        ot = wp.tile([128, H], BF16, tag="po")
                        nc.vector.tensor_scalar_mul(ot[:], ps[:], norm[f"odeg_{r}"][:, k:k + 1])
                        nc.sync.dma_start(feat_in["1" + r][sl_, :], ot[:])
                    nc.gpsimd.collective_compute("AllGather", ALU.bypass, replica_groups=RG,
                                                 ins=[feat_in["1" + r][:]], outs=[feat_pair["1" + r][:]])
            gcn_layer(1, None)
            with tc.tile_pool(name="zw", bufs=1) as wp:
                zr = wp.tile([128, H], F32, tag="zr")
                nc.vector.memset(zr[:], 0.0)
                for pk in ("p1", "p2"):
                    nc.sync.dma_start(p_pair[pk][PAIRZ:PAIRZ + 64, :], zr[:])
                    nc.gpsimd.collective_compute("AllGather", ALU.bypass, replica_groups=RG,
                                                 ins=[p_in[pk][:]], outs=[p_pair[pk][0:PAIRZ, :]])
                if DBG:
                    nc.sync.dma_start(dbg["p1"][:], p_pair["p1"][:])
                    nc.sync.dma_start(dbg["p2"][:], p_pair["p2"][:])
                    nc.sync.dma_start(dbg["f0sim"][:], feat_pair["0sim"][:])

            # ---------- classifier ----------
            # z = p1[s] + p2[d] (edge-major, bias-free: BatchNorm cancels cls_b1);
            # batch stats via ones-matmul accumulation + AllReduce; then
            # transpose + fused affine/relu + W2.
            with tc.tile_pool(name="cw", bufs=3) as wp, \
                 tc.tile_pool(name="cz", bufs=1) as zp, \
                 tc.tile_pool(name="cg", bufs=2) as gp, \
                 tc.tile_pool(name="cstat", bufs=1, space="PSUM") as sp, \
                 tc.tile_pool(name="cp", bufs=2, space="PSUM") as pp:
                outst = zp.tile([128, CC * 2], F32, tag="outst")
                sums = zp.tile([128, CC], F32, tag="sums")
                sqs = zp.tile([128, CC], F32, tag="sqs")

                def cls_gather(koff, nb, bi, qs, qd):
                    gs = gp.tile([128, nb, H], F32, tag="cgs")
                    gd = gp.tile([128, nb, H], F32, tag="cgd")
                    s_base = p_pair["p1"][:, 0:H] if bi < 2 else p_pair["p1"][:, H:2 * H]
                    d_base = p_pair["p2"][:, 0:H] if bi % 2 == 0 else p_pair["p2"][:, H:2 * H]
                    nc.gpsimd.dma_gather(gs[:], s_base, idxt["cs_idx"][:, koff * 8:(koff + nb) * 8],
                                         nb * 128, nb * 128, H, elem_step=2 * H,
                                         single_packet=False, queue_num=qs)
                    nc.gpsimd.dma_gather(gd[:], d_base, idxt["cd_idx"][:, koff * 8:(koff + nb) * 8],
                                         nb * 128, nb * 128, H, elem_step=2 * H,
                                         single_packet=False, queue_num=qd)
                    return gs, gd

                koff = 0
                for bi in range(4):
                    nb = meta["CBK"][bi]
                    gs, gd = cls_gather(koff, nb, bi, bi, (bi + 2) % 4)
                    for j in range(nb):
                        k = koff + j
                        wz = wp.tile([128, 128], F32, tag="wz")
                        nc.vector.tensor_add(wz[:], gs[:, j, :], gd[:, j, :])
                        pst = pp.tile([128, 128], F32, tag="ctr")
                        nc.tensor.transpose(pst[:], wz[:], ident[:])
                        zt = wp.tile([128, 128], F32, tag="zt")
                        nc.scalar.activation(zt[:], pst[:], A.Copy, accum_out=sums[:, k:k + 1])
                        sq = wp.tile([128, 128], F32, tag="csq")
                        nc.scalar.activation(sq[:], zt[:], A.Square, accum_out=sqs[:, k:k + 1])
                        nc.sync.dma_start(zt_dram[:, k * 128:(k + 1) * 128], zt[:])
                    koff += nb
                s1 = wp.tile([128, 1], F32, tag="s1")
                nc.vector.tensor_reduce(s1[:], sums[:], mybir.AxisListType.X, ALU.add)
                s2 = wp.tile([128, 1], F32, tag="s2")
                nc.vector.tensor_reduce(s2[:], sqs[:], mybir.AxisListType.X, ALU.add)
                stt = wp.tile([128, 2], F32, tag="stt")
                nc.vector.tensor_copy(stt[:, 0:1], s1[:])
                nc.vector.tensor_copy(stt[:, 1:2], s2[:])
                nc.sync.dma_start(st_in[:], stt[:])
                nc.gpsimd.collective_compute("AllReduce", ALU.add, replica_groups=RG,
                                             ins=[st_in[:]], outs=[st_out[:]])
                stg = wp.tile([128, 2], F32, tag="stg")
                nc.sync.dma_start(stg[:], st_out[:])
                if DBG:
                    nc.sync.dma_start(dbg["st"][:], stg[:])
                stc = wp.tile([128, 2], F32, tag="stc")
                nc.vector.tensor_scalar_mul(stc[:], stg[:], 1.0 / ESUB)
                var = wp.tile([128, 1], F32, tag="var")
                nc.vector.tensor_mul(var[:], stc[:, 0:1], stc[:, 0:1])
                nc.vector.tensor_tensor(out=var[:], in0=stc[:, 1:2], in1=var[:], op=ALU.subtract)
                epsc = wp.tile([128, 1], F32, tag="eps")
                nc.vector.memset(epsc[:], 1e-5)
                sd_ = wp.tile([128, 1], F32, tag="sd")
                nc.scalar.activation(sd_[:], var[:], A.Sqrt, bias=epsc[:, :])
                inv = wp.tile([128, 1], F32, tag="inv")
                nc.vector.reciprocal(inv[:], sd_[:])
                Acol = wp.tile([128, 1], F32, tag="Ac")
                nc.vector.tensor_mul(Acol[:], consts["bngb"][:, 0:1], inv[:])
                Ccol = wp.tile([128, 1], F32, tag="Cc")
                nc.vector.tensor_mul(Ccol[:], stc[:, 0:1], Acol[:])
                nc.vector.tensor_tensor(out=Ccol[:], in0=consts["bngb"][:, 1:2], in1=Ccol[:], op=ALU.subtract)
                for k in range(CC):
                    zt = wp.tile([128, 128], F32, tag="zt2")
                    nc.sync.dma_start(zt[:], zt_dram[:, k * 128:(k + 1) * 128])
                    zn = wp.tile([128, 128], BF16, tag="zn")
                    nc.scalar.activation(zn[:], zt[:], A.Relu, bias=Ccol[:, :], scale=Acol[:, :])
                    pso = pp.tile([128, 2], F32, tag="co")
                    nc.tensor.matmul(pso[:], zn[:], consts["W2T"][:], start=True, stop=True)
                    nc.vector.tensor_add(outst[:, 2 * k:2 * k + 2], pso[:], consts["b2bias"][:])
                nc.sync.dma_start(out[:], outst[:])
    nc.compile()
    return nc


_CACHE = {}


def kernel(**inputs):
    in_maps, out_rows, meta = host_prep(inputs)
    key = (meta["TOTS"], meta["CC"], tuple(meta["gcn"]["sim"]["KE"]),
           tuple(meta["gcn"]["sim"]["KO"]), tuple(meta["gcn"]["user"]["KE"]),
           tuple(meta["gcn"]["user"]["KO"]), tuple(meta["CBK"]))
    if key not in _CACHE:
        _CACHE[key] = build_program(meta)
    nc = _CACHE[key]
    import os
    trace = bool(os.environ.get("KTRACE"))
    res = run_bass_kernel_spmd(nc, in_maps, list(range(NC)), trace=trace)
    if trace and res.exec_time_ns is not None:
        print(f"HW exec time: {res.exec_time_ns} ns")
    outp = np.zeros((ESUB, 2), np.float32)
    CC = meta["CC"]
    for c in range(NC):
        o = np.asarray(res.results[c]["out"], np.float32)
        o = o.reshape(128, CC, 2).transpose(1, 0, 2).reshape(CC * 128, 2)
        rows = out_rows[c]
        m = rows >= 0
        outp[rows[m]] = o[m]
    return outp



# revision 34
# speedup vs baseline: 1.1141x; 1.1141x over previous
"""Trainium2 Bass kernel: LSTM url encoder + 2-layer hetero GCN + edge classifier,
sharded over 8 NeuronCores (nodes re-labeled to per-core slots with identical
length profile; message passing via dma_gather + 0/1-indicator PSUM matmuls;
feature tables replicated with AllGather; BatchNorm stats via AllReduce)."""
import sys, types
import numpy as np

try:
    from trn_agent_boot.trn_boot import _ntff_profile_via_ctypes
    _m = types.ModuleType("antenv.axon_hooks")
    _m.get_axon_ntff_profile_hook = lambda: _ntff_profile_via_ctypes('/opt/axon/libaxon_pjrt.so')
    _m.set_axon_ntff_profile_hook = lambda h: None
    sys.modules.setdefault("antenv.axon_hooks", _m)
except Exception:
    pass

import concourse.bass as bass
import concourse.tile as tile
from concourse import bacc, mybir
from concourse.bass_utils import run_bass_kernel_spmd
from concourse.library_config import mlp
from concourse.masks import make_identity
import ml_dtypes

F32 = mybir.dt.float32
BF16 = mybir.dt.bfloat16
NPBF = ml_dtypes.bfloat16
I8 = mybir.dt.int8
I16 = mybir.dt.int16
ALU = mybir.AluOpType

NC = 8
N, L, E, H = 50000, 32, 64, 128
NSLOT = 6656
NBLK, BW = 13, 512
NTILE = NSLOT // 128
NGLOB = NC * NSLOT
ZROW = NGLOB
PAIRZ = NGLOB // 2     # zeroed pad pair-row in the p1/p2 tables
ESUB = 200000
ESH = ESUB // NC
GRP = 2                # dst tiles per merged gather group


def _wrap_idx(iv):
    n = len(iv)
    w = np.zeros((128, n // 16), np.int16)
    blk = iv.reshape(n // 16, 16).T
    for g in range(8):
        w[16 * g:16 * (g + 1)] = blk
    return w


def host_prep(inp):
    lens = np.asarray(inp["inputs_sm"]).astype(np.int64).sum(1)
    ids = np.asarray(inp["inputs_s"]).astype(np.int64)
    order = np.argsort(lens, kind="stable")
    g_of = np.zeros(N, np.int64)
    slot_node = np.full((NC, NSLOT), -1, np.int64)
    lens_sorted = lens[order]
    prof = []
    per_core_pos = 0
    idx0 = 0
    while idx0 < N:
        l = int(lens_sorted[idx0])
        idx1 = idx0
        while idx1 < N and lens_sorted[idx1] == l:
            idx1 += 1
        nodes = order[idx0:idx1]
        m = -(-(idx1 - idx0) // NC)
        for j, nd in enumerate(nodes):
            c, p = j % NC, per_core_pos + j // NC
            slot_node[c, p] = nd
            g_of[nd] = c * NSLOT + p
        prof.extend([l] * m)
        per_core_pos += m
        idx0 = idx1
    prof.extend([prof[-1]] * (NSLOT - per_core_pos))
    slot_len = np.array(prof, np.int64)
    steps_chunk = [int(slot_len[min(k * 128 + 127, NSLOT - 1)]) for k in range(NTILE)]
    steps_blk = [max(steps_chunk[b * 4:(b + 1) * 4]) for b in range(NBLK)]
    harvests = []
    for b in range(NBLK):
        ev = []
        ls = slot_len[b * BW:(b + 1) * BW]
        for l in np.unique(ls):
            w = np.where(ls == l)[0]
            ev.append((int(l) - 1, int(w[0]), int(w[-1]) + 1))
        harvests.append(ev)
    TOTS = sum(steps_blk)
    meta = dict(steps_blk=steps_blk, harvests=harvests, TOTS=TOTS)

    f32 = lambda a: np.ascontiguousarray(a, np.float32)
    bf16 = lambda a: np.ascontiguousarray(np.asarray(a, np.float32), NPBF)
    # gate column order [i, f, o, g] (PyTorch i,f,g,o reordered)
    GPERM = np.r_[0:E, E:2 * E, 3 * E:4 * E, 2 * E:3 * E]
    emb_url = f32(inp["emb_url"])
    shared = {}
    GSC = np.ones(4 * E, np.float32)
    GSC[:3 * E] = 0.5          # tanh(0.5*x) for i,f,o; tanh(x) for g
    for suf in ("f", "b"):
        ep = emb_url @ f32(inp["Wih_" + suf]).T + f32(inp["b_" + suf])[None, :]
        shared[f"embproj_{suf}"] = bf16(ep[:, GPERM] * GSC[None, :])
    whhp = np.zeros((128, 4 * E), np.float32)
    whhp[0:E] = 0.5 * f32(inp["Whh_f"]).T[:, GPERM] * GSC[None, :]
    whhp[E:128] = 0.5 * f32(inp["Whh_b"]).T[:, GPERM] * GSC[None, :]
    shared["whhT"] = bf16(whhp)
    shared["fcWT"] = bf16(0.5 * f32(inp["fc_W"]).T)
    fcb = np.zeros((128, 1), np.float32)
    fcb[0:E, 0] = f32(inp["fc_b"])
    shared["fcb"] = fcb
    slope = np.ones((128, 1), np.float32)
    slope[0:E, 0] = 0.01
    shared["slope_a"] = slope
    for r in ("sim", "user"):
        shared[f"W0{r}a"] = bf16(f32(inp[f"gcn0_{r}_W"])[:128])
        shared[f"W0{r}b"] = bf16(f32(inp[f"gcn0_{r}_W"])[128:])
        shared[f"W1{r}"] = bf16(f32(inp[f"gcn1_{r}_W"]))
    # cls_b1 is dropped: BatchNorm with batch stats cancels any pre-BN bias.
    shared["W1aT"] = f32(inp["cls_W1"])[:, :H].T.copy()
    shared["W1bT"] = f32(inp["cls_W1"])[:, H:].T.copy()
    shared["bngb"] = np.stack([f32(inp["bn_g"]), f32(inp["bn_b"])], 1)
    shared["W2T"] = bf16(f32(inp["cls_W2"]).T)
    shared["b2bias"] = np.tile(f32(inp["cls_b2"])[None, :], (128, 1))
    for lyr in (0, 1):
        bb = f32(inp[f"gcn{lyr}_sim_b"]) + f32(inp[f"gcn{lyr}_user_b"])
        shared[f"biasL{lyr}"] = np.tile(bb[None, :], (128, 1))

    edges, deg = {}, {}
    for r in ("sim", "user"):
        src = g_of[np.asarray(inp[r + "_src"]).astype(np.int64)]
        dst = g_of[np.asarray(inp[r + "_dst"]).astype(np.int64)]
        od = np.zeros(NGLOB, np.float32); np.add.at(od, src, 1.0)
        idg = np.zeros(NGLOB, np.float32); np.add.at(idg, dst, 1.0)
        deg[r] = (np.maximum(od, 1.0), np.maximum(idg, 1.0))
        edges[r] = (src, dst)

    # GCN edges: chunks of 128 edges per dst tile, split by src parity so a
    # single bf16 pair-table [NGLOB//2, 2H] serves int16 indices (idx = src>>1).
    groups = [list(range(g, min(g + GRP, NTILE))) for g in range(0, NTILE, GRP)]
    gcn = {}
    for r in ("sim", "user"):
        src, dst = edges[r]
        core = dst // NSLOT
        dl = dst % NSLOT
        tt = dl // 128
        by = [[[[], []] for _ in range(NTILE)] for _ in range(NC)]
        for c in range(NC):
            m = np.where(core == c)[0]
            for e in m:
                by[c][tt[e]][int(src[e]) & 1].append((int(src[e]), int(dl[e]) % 128))
        KE = [max(1, max(-(-len(by[c][t][0]) // 128) for c in range(NC))) for t in range(NTILE)]
        KO = [max(1, max(-(-len(by[c][t][1]) // 128) for c in range(NC))) for t in range(NTILE)]
        gcn[r] = dict(KE=KE, KO=KO, by=by)
    meta["gcn"] = {r: dict(KE=gcn[r]["KE"], KO=gcn[r]["KO"]) for r in ("sim", "user")}
    meta["groups"] = groups

    es = g_of[np.asarray(inp["esub_src"]).astype(np.int64)]
    ed = g_of[np.asarray(inp["esub_dst"]).astype(np.int64)]
    cls_bk = []
    for c in range(NC):
        sl = slice(c * ESH, (c + 1) * ESH)
        s_c, d_c, gi = es[sl], ed[sl], np.arange(c * ESH, (c + 1) * ESH)
        bks = []
        for bi in range(4):
            m = ((s_c & 1) * 2 + (d_c & 1)) == bi
            bks.append((s_c[m], d_c[m], gi[m]))
        cls_bk.append(bks)
    CBK = [max(1, max(-(-len(cls_bk[c][bi][0]) // 128) for c in range(NC))) for bi in range(4)]
    CC = sum(CBK)
    meta.update(CC=CC, CBK=CBK)

    in_maps, out_rows = [], []
    for c in range(NC):
        m = dict(shared)
        ohc = np.zeros((2, TOTS, 128, BW), NPBF)
        off = 0
        for b in range(NBLK):
            nd = slot_node[c, b * BW:(b + 1) * BW]
            ln = slot_len[b * BW:(b + 1) * BW]
            real = nd >= 0
            idsF = np.zeros((BW, L), np.int64)
            idsB = np.zeros((BW, L), np.int64)
            idsF[real] = ids[nd[real]]
            for j in np.where(real)[0]:
                lj = int(lens[nd[j]])
                idsB[j, :lj] = ids[nd[j], :lj][::-1]
            for s in range(steps_blk[b]):
                w = np.where((s < ln) & real)[0]
                ohc[0, off, idsF[w, s], w] = 1
                ohc[1, off, idsB[w, s], w] = 1
                off += 1
        m["oh_f"] = np.ascontiguousarray(ohc[0].reshape(TOTS * 128, BW))
        m["oh_b"] = np.ascontiguousarray(ohc[1].reshape(TOTS * 128, BW))
        for name, key in (("ohcat", "inputs_c"), ("ohcou", "inputs_co"), ("ohsl", "inputs_sl")):
            o = np.zeros((128, NSLOT), NPBF)
            nd = slot_node[c]
            real = nd >= 0
            vals = np.zeros(NSLOT, np.int64)
            vals[real] = np.asarray(inp[key]).astype(np.int64)[nd[real], 0]
            o[vals, np.arange(NSLOT)] = 1
            m[name] = o
        m["ecat"] = np.zeros((128, E), NPBF); m["ecat"][:101] = bf16(inp["emb_cat"])
        m["ecou"] = np.zeros((128, E), NPBF); m["ecou"][:92] = bf16(inp["emb_country"])
        m["esl"] = np.zeros((128, E), NPBF); m["esl"][:6] = bf16(inp["emb_sl"])
        sl = slice(c * NSLOT, (c + 1) * NSLOT)
        for r in ("sim", "user"):
            od, idg = deg[r]
            m[f"odeg_{r}"] = np.ascontiguousarray(od[sl].reshape(NTILE, 128).T)
            m[f"ideg_{r}"] = np.ascontiguousarray(idg[sl].reshape(NTILE, 128).T)
        for r in ("sim", "user"):
            KE, KO, by = gcn[r]["KE"], gcn[r]["KO"], gcn[r]["by"]
            idx_all, ind_cols = [], []
            for grp in groups:
                for parity, KX in ((0, KE), (1, KO)):
                    for t in grp:
                        KK = KX[t]
                        lst = by[c][t][parity]
                        arr = np.zeros((KK * 128, 2), np.int64)
                        ok = np.zeros(KK * 128, bool)
                        if lst:
                            a = np.array(lst, np.int64)
                            arr[:len(a)] = a
                            ok[:len(a)] = True
                        iv = (arr[:, 0] >> 1).astype(np.int16)
                        iv[~ok] = 0
                        idx_all.append(iv)
                        mm = np.zeros((KK * 128, 128), np.int8)
                        mm[np.where(ok)[0], arr[ok, 1]] = 1
                        ind_cols.append(mm.reshape(KK, 128, 128).transpose(1, 0, 2).reshape(128, KK * 128))
            m[f"gidx_{r}"] = _wrap_idx(np.concatenate(idx_all))
            m[f"gind_{r}"] = np.ascontiguousarray(np.concatenate(ind_cols, 1))
        rows = np.full(CC * 128, -1, np.int64)
        sidx = np.zeros(CC * 128, np.int64); didx = np.zeros(CC * 128, np.int64)
        o = 0
        for bi in range(4):
            s_c, d_c, gi = cls_bk[c][bi]
            nn_ = len(s_c)
            sidx[o:o + nn_] = s_c; didx[o:o + nn_] = d_c; rows[o:o + nn_] = gi
            sidx[o + nn_:o + CBK[bi] * 128] = 2 * PAIRZ; didx[o + nn_:o + CBK[bi] * 128] = 2 * PAIRZ
            o += CBK[bi] * 128
        m["cs_idx"] = _wrap_idx((sidx >> 1).astype(np.int16))
        m["cd_idx"] = _wrap_idx((didx >> 1).astype(np.int16))
        out_rows.append(rows)
        in_maps.append(m)
    return in_maps, out_rows, meta


def build_program(meta):
    nc = bacc.Bacc("TRN2", target_bir_lowering=False, debug=False, num_devices=NC,
                   num_swdge_queues=4)
    TOTS, CC = meta["TOTS"], meta["CC"]
    ein = lambda n, s, d=F32: nc.dram_tensor(n, s, d, kind="ExternalInput")
    x = {}
    for d in ("f", "b"):
        x["embproj_" + d] = ein("embproj_" + d, [128, 4 * E], BF16)
        x["oh_" + d] = ein("oh_" + d, [TOTS * 128, BW], BF16)
    for nm, sh, dt in (("whhT", [128, 4 * E], BF16),
                       ("fcWT", [2 * E, E], BF16), ("fcb", [128, 1], F32),
                       ("slope_a", [128, 1], F32),
                       ("ohcat", [128, NSLOT], BF16), ("ohcou", [128, NSLOT], BF16),
                       ("ohsl", [128, NSLOT], BF16), ("ecat", [128, E], BF16),
                       ("ecou", [128, E], BF16), ("esl", [128, E], BF16),
                       ("W1aT", [H, H], F32), ("W1bT", [H, H], F32),
                       ("W2T", [H, 2], BF16),
                       ("bngb", [128, 2], F32),
                       ("b2bias", [128, 2], F32), ("biasL0", [128, H], F32),
                       ("biasL1", [128, H], F32),
                       ("cs_idx", [128, CC * 8], I16), ("cd_idx", [128, CC * 8], I16)):
        x[nm] = ein(nm, sh, dt)
    for r in ("sim", "user"):
        x[f"W0{r}a"] = ein(f"W0{r}a", [H, H], BF16)
        x[f"W0{r}b"] = ein(f"W0{r}b", [H, H], BF16)
        x[f"W1{r}"] = ein(f"W1{r}", [H, H], BF16)
    for r in ("sim", "user"):
        TK = sum(meta["gcn"][r]["KE"]) + sum(meta["gcn"][r]["KO"])
        x[f"gidx_{r}"] = ein(f"gidx_{r}", [128, TK * 8], I16)
        x[f"gind_{r}"] = ein(f"gind_{r}", [128, TK * 128], I8)
        x[f"odeg_{r}"] = ein(f"odeg_{r}", [128, NTILE])
        x[f"ideg_{r}"] = ein(f"ideg_{r}", [128, NTILE])
    out = nc.dram_tensor("out", [128, CC * 2], F32, kind="ExternalOutput")

    feat_in = {k: nc.dram_tensor(f"fin_{k}", [NSLOT, H], BF16) for k in ("0sim", "0user", "1sim", "1user")}
    feat_pair = {k: nc.dram_tensor(f"fp_{k}", [NGLOB // 2, 2 * H], BF16, addr_space="Shared")
                 for k in ("0sim", "0user", "1sim", "1user")}
    p_in = {k: nc.dram_tensor(f"pin_{k}", [NSLOT, H], F32) for k in ("p1", "p2")}
    p_pair = {k: nc.dram_tensor(f"pp_{k}", [PAIRZ + 64, 2 * H], F32, addr_space="Shared")
              for k in ("p1", "p2")}
    zt_dram = nc.dram_tensor("zt_dram", [128, CC * 128], F32)
    st_in = nc.dram_tensor("st_in", [128, 2], F32)
    st_out = nc.dram_tensor("st_out", [128, 2], F32, addr_space="Shared")
    RG = [list(range(NC))]
    A = mybir.ActivationFunctionType
    groups = meta["groups"]
    import os
    DBG = bool(os.environ.get("KDEBUG"))
    dbg = {}
    if DBG:
        dbg["f0sim"] = nc.dram_tensor("dbg_f0sim", [NGLOB // 2, 2 * H], BF16, kind="ExternalOutput")
        dbg["h1T"] = nc.dram_tensor("dbg_h1T", [128, NSLOT], BF16, kind="ExternalOutput")
        dbg["p1"] = nc.dram_tensor("dbg_p1", [PAIRZ + 64, 2 * H], F32, kind="ExternalOutput")
        dbg["p2"] = nc.dram_tensor("dbg_p2", [PAIRZ + 64, 2 * H], F32, kind="ExternalOutput")
        dbg["st"] = nc.dram_tensor("dbg_st", [128, 2], F32, kind="ExternalOutput")

    with tile.TileContext(nc) as tc:
        nc.gpsimd.load_library(mlp)
        with tc.tile_pool(name="const", bufs=1) as cpool, \
             tc.tile_pool(name="stage", bufs=1) as spool:
            ident = cpool.tile([128, 128], F32)
            make_identity(nc, ident)
            identb = cpool.tile([128, 128], BF16, tag="identb")
            nc.vector.tensor_copy(identb[:], ident[:])
            onecol = cpool.tile([128, 1], F32, tag="onecol")
            nc.vector.memset(onecol[:], 1.0)
            hsp_cm = tc.tile_pool(name="hstage", bufs=1)
            hsp = hsp_cm.__enter__()
            LSTM_ONLY = {"embproj_f", "embproj_b", "whhT", "fcWT", "fcb",
                         "slope_a", "ecat", "ecou", "esl", "W0sima", "W0simb",
                         "W0usera", "W0userb"}
            consts = {}
            for nm in ("embproj_f", "embproj_b", "whhT",
                       "fcWT", "fcb", "slope_a", "ecat", "ecou", "esl",
                       "W0sima", "W0simb", "W0usera", "W0userb",
                       "W1sim", "W1user", "W1aT", "W1bT", "W2T",
                       "bngb", "b2bias", "biasL0", "biasL1"):
                pool_ = hsp if nm in LSTM_ONLY else cpool
                t = pool_.tile(list(x[nm].shape), x[nm].dtype, tag=nm)
                nc.sync.dma_start(t[:], x[nm][:])
                consts[nm] = t
            idxt = {}
            for nm in ("gidx_sim", "gidx_user", "cs_idx", "cd_idx"):
                t = cpool.tile(list(x[nm].shape), I16, tag="i" + nm)
                nc.sync.dma_start(t[:], x[nm][:])
                idxt[nm] = t
            norm = {}
            for r in ("sim", "user"):
                for kind in ("odeg", "ideg"):
                    t = cpool.tile([128, NTILE], F32, tag=f"d{kind}{r}")
                    nc.sync.dma_start(t[:], x[f"{kind}_{r}"][:])
                    s = cpool.tile([128, NTILE], F32, tag=f"s{kind}{r}")
                    nc.scalar.sqrt(s[:], t[:])
                    rv = cpool.tile([128, NTILE], F32, tag=f"r{kind}{r}")
                    nc.vector.reciprocal(rv[:], s[:])
                    norm[f"{kind}_{r}"] = rv

            hhT = hsp.tile([128, NSLOT], BF16, tag="bigh")
            # ---------- LSTM ----------
            # Packed layout: partitions 0:64 = forward dir, 64:128 = backward.
            # h' = 2h (fold into whhT/fcWT at host); sigmoid via tanh:
            # sigma(x) = 0.5*tanh(0.5x) + 0.5. Gate bias folded into embproj.
            off_blk = [0]
            for b in range(NBLK):
                off_blk.append(off_blk[-1] + meta["steps_blk"][b])
            with tc.tile_pool(name="lwork", bufs=3) as wp, \
                 tc.tile_pool(name="lstate", bufs=2) as lsp, \
                 tc.tile_pool(name="lps", bufs=1, space="PSUM") as pp:
                def lstm_step(b, s, par, S_b, h_b):
                    base = (off_blk[b] + s) * 128
                    ohf = wp.tile([128, BW], BF16, tag=f"ohf{par}")
                    nc.sync.dma_start(ohf[:], x["oh_f"][base:base + 128, :])
                    ohb = wp.tile([128, BW], BF16, tag=f"ohb{par}")
                    nc.sync.dma_start(ohb[:], x["oh_b"][base:base + 128, :])
                    ps3 = pp.tile([128, 3 * BW], F32, tag=f"ifo{par}")
                    psg = pp.tile([128, BW], F32, tag=f"gg{par}")
                    outs = [ps3[:, 0:BW], ps3[:, BW:2 * BW], ps3[:, 2 * BW:3 * BW], psg[:]]
                    for j in range(4):
                        g64 = slice(j * E, (j + 1) * E)
                        o = outs[j]
                        nc.tensor.matmul(o[0:E, :], consts["embproj_f"][:, g64], ohf[:], start=True, stop=False)
                        nc.tensor.matmul(o[E:128, :], consts["embproj_b"][:, g64], ohb[:], start=True, stop=False)
                    for j in range(4):
                        g64 = slice(j * E, (j + 1) * E)
                        o = outs[j]
                        nc.tensor.matmul(o[0:E, :], consts["whhT"][0:E, g64], h_b[0:E, :], start=False, stop=True)
                        nc.tensor.matmul(o[E:128, :], consts["whhT"][E:128, g64], h_b[E:128, :], start=False, stop=True)
                    tifo = wp.tile([128, 3 * BW], BF16, tag=f"tifo{par}")
                    nc.scalar.activation(tifo[:], ps3[:], A.Tanh)
                    tg = wp.tile([128, BW], BF16, tag=f"tg{par}")
                    nc.scalar.activation(tg[:], psg[:], A.Tanh)
                    A2 = wp.tile([128, BW], F32, tag=f"A2{par}")
                    nc.vector.scalar_tensor_tensor(A2[:], tifo[:, BW:2 * BW], 1.0, S_b[:], ALU.add, ALU.mult)
                    B2 = wp.tile([128, BW], BF16, tag=f"B2{par}")
                    nc.vector.scalar_tensor_tensor(B2[:], tifo[:, 0:BW], 1.0, tg[:], ALU.add, ALU.mult)
                    nc.vector.scalar_tensor_tensor(S_b[:], A2[:], 0.5, B2[:], ALU.mult, ALU.add)
                    tc_ = wp.tile([128, BW], BF16, tag=f"tc{par}")
                    nc.scalar.activation(tc_[:], S_b[:], A.Tanh, scale=0.5)
                    nc.vector.scalar_tensor_tensor(h_b[:], tifo[:, 2 * BW:3 * BW], 1.0, tc_[:], ALU.add, ALU.mult)
                    for (sd, c0, c1) in meta["harvests"][b]:
                        if sd == s:
                            nc.vector.tensor_copy(hhT[:, b * BW + c0:b * BW + c1], h_b[:, c0:c1])

                bpairs = [tuple(range(b, min(b + 2, NBLK))) for b in range(0, NBLK, 2)]
                for pair in bpairs:
                    st = {}
                    for par, b in enumerate(pair):
                        S_b = lsp.tile([128, BW], F32, tag=f"S{par}")
                        h_b = lsp.tile([128, BW], BF16, tag=f"h{par}")
                        nc.vector.memset(S_b[:], 0.0)
                        nc.vector.memset(h_b[:], 0.0)
                        st[b] = (S_b, h_b)
                    smax = max(meta["steps_blk"][b] for b in pair)
                    for s in range(smax):
                        for par, b in enumerate(pair):
                            if s < meta["steps_blk"][b]:
                                lstm_step(b, s, par, *st[b])

            # ---------- fc + embeds + feat0 proj ----------
            h0a = hsp.tile([128, NSLOT], BF16, tag="big2")
            h0b = hsp.tile([128, NSLOT], BF16, tag="big3")
            with tc.tile_pool(name="fwork", bufs=3) as wp, \
                 tc.tile_pool(name="fps", bufs=2, space="PSUM") as pp:
                for b in range(NBLK):
                    sl_ = slice(b * BW, (b + 1) * BW)
                    psa = pp.tile([128, BW], F32, tag="psa")
                    psb = pp.tile([128, BW], F32, tag="psb")
                    nc.tensor.matmul(psa[0:E, :], consts["fcWT"][:], hhT[:, sl_], start=True, stop=True)
                    for (ohn, etab, ps_, dr) in (("ohcat", "ecat", psa, slice(E, 128)),
                                                 ("ohcou", "ecou", psb, slice(0, E)),
                                                 ("ohsl", "esl", psb, slice(E, 128))):
                        ohp = wp.tile([128, BW], BF16, tag="oh" + ohn[2:])
                        nc.sync.dma_start(ohp[:], x[ohn][:, sl_])
                        nc.tensor.matmul(ps_[dr, :], consts[etab][:], ohp[:], start=True, stop=True)
                    za = wp.tile([128, BW], F32, tag="za")
                    nc.scalar.activation(za[:], psa[:], A.Identity, bias=consts["fcb"][:, :])
                    nc.vector.scalar_tensor_tensor(h0a[:, sl_], za[:], consts["slope_a"][:, 0:1], za[:], ALU.mult, ALU.max)
                    nc.vector.tensor_copy(h0b[:, sl_], psb[:])
                for r in ("sim", "user"):
                    for k in range(NTILE):
                        sl_ = slice(k * 128, (k + 1) * 128)
                        ps = pp.tile([128, H], F32, tag="proj")
                        nc.tensor.matmul(ps[:], h0a[:, sl_], consts[f"W0{r}a"][:], start=True, stop=False)
                        nc.tensor.matmul(ps[:], h0b[:, sl_], consts[f"W0{r}b"][:], start=False, stop=True)
                        ot = wp.tile([128, H], BF16, tag="po")
                        nc.vector.tensor_scalar_mul(ot[:], ps[:], norm[f"odeg_{r}"][:, k:k + 1])
                        nc.sync.dma_start(feat_in["0" + r][sl_, :], ot[:])
                    nc.gpsimd.collective_compute("AllGather", ALU.bypass, replica_groups=RG,
                                                 ins=[feat_in["0" + r][:]], outs=[feat_pair["0" + r][:]])
            hsp_cm.__exit__(None, None, None)

            # ---------- GCN ----------
            # Gathers use pair-tables [NGLOB//2, 2H]: idx = src >> 1 (int16-safe),
            # elem_step = 2 rows, base column offset selects src parity. Chunks are
            # parity-homogeneous; gather calls merged over GRP dst tiles, spread
            # over 4 SWDGE queues (parallel Q7 desc-gen on 4 core pairs).
            def gcn_layer(lyr, h_next_T):
                with tc.tile_pool(name=f"gw{lyr}", bufs=2) as wp, \
                     tc.tile_pool(name=f"gg{lyr}", bufs=2) as gp, \
                     tc.tile_pool(name=f"gp{lyr}", bufs=2, space="PSUM") as pp, \
                     tc.tile_pool(name=f"gt{lyr}", bufs=1, space="PSUM") as pt:
                    idx_off = {r: 0 for r in ("sim", "user")}
                    ind_off = {r: 0 for r in ("sim", "user")}
                    for gi, grp in enumerate(groups):
                        pools = {}
                        for qb, r in ((0, "sim"), (2, "user")):
                            KE = [meta["gcn"][r]["KE"][t] for t in grp]
                            KO = [meta["gcn"][r]["KO"][t] for t in grp]
                            KEg, KOg = sum(KE), sum(KO)
                            pe = gp.tile([128, KEg, H], BF16, tag=f"pe{r}")
                            po = gp.tile([128, KOg, H], BF16, tag=f"po{r}")
                            io = idx_off[r]
                            nc.gpsimd.dma_gather(pe[:], feat_pair[f"{lyr}{r}"][:, 0:H],
                                                 idxt[f"gidx_{r}"][:, io // 16:(io + KEg * 128) // 16],
                                                 KEg * 128, KEg * 128, H, elem_step=2 * H,
                                                 single_packet=False, queue_num=qb)
                            nc.gpsimd.dma_gather(po[:], feat_pair[f"{lyr}{r}"][:, H:2 * H],
                                                 idxt[f"gidx_{r}"][:, (io + KEg * 128) // 16:(io + (KEg + KOg) * 128) // 16],
                                                 KOg * 128, KOg * 128, H, elem_step=2 * H,
                                                 single_packet=False, queue_num=qb + 1)
                            idx_off[r] = io + (KEg + KOg) * 128
                            ind = gp.tile([128, (KEg + KOg) * 128], BF16, tag=f"ind{r}")
                            nc.gpsimd.dma_start(ind[:], x[f"gind_{r}"][:, ind_off[r]:ind_off[r] + (KEg + KOg) * 128])
                            ind_off[r] += (KEg + KOg) * 128
                            pools[r] = (pe, po, ind, KE, KO, KEg)
                        for ti, t in enumerate(grp):
                            res = {}
                            for r in ("sim", "user"):
                                pe, po, ind, KE, KO, KEg = pools[r]
                                eb, ob = sum(KE[:ti]), sum(KO[:ti])
                                ps = pp.tile([128, H], F32, tag="sc" + r)
                                K = KE[ti] + KO[ti]
                                kk = 0
                                for j in range(KE[ti]):
                                    nc.tensor.matmul(ps[:], ind[:, (eb + j) * 128:(eb + j + 1) * 128],
                                                     pe[:, eb + j, :], start=(kk == 0), stop=(kk == K - 1))
                                    kk += 1
                                for j in range(KO[ti]):
                                    nc.tensor.matmul(ps[:], ind[:, (KEg + ob + j) * 128:(KEg + ob + j + 1) * 128],
                                                     po[:, ob + j, :], start=(kk == 0), stop=(kk == K - 1))
                                    kk += 1
                                res[r] = ps
                            t1 = wp.tile([128, H], F32, tag="e1")
                            nc.vector.tensor_scalar_mul(t1[:], res["sim"][:], norm["ideg_sim"][:, t:t + 1])
                            t2 = wp.tile([128, H], F32, tag="e2")
                            nc.vector.scalar_tensor_tensor(t2[:], res["user"][:], norm["ideg_user"][:, t:t + 1],
                                                           t1[:], ALU.mult, ALU.add)
                            t3 = wp.tile([128, H], F32, tag="e3")
                            nc.vector.tensor_add(t3[:], t2[:], consts[f"biasL{lyr}"][:])
                            if h_next_T is not None:
                                hrow = wp.tile([128, H], BF16, tag="e4")
                                nc.vector.scalar_tensor_tensor(hrow[:], t3[:], 0.01, t3[:], ALU.mult, ALU.max)
                                ps2 = pt.tile([128, H], BF16, tag="tr")
                                nc.tensor.transpose(ps2[:], hrow[:], identb[:])
                                nc.vector.tensor_copy(h_next_T[:, t * 128:(t + 1) * 128], ps2[:])
                            else:
                                hrow = wp.tile([128, H], F32, tag="e4f")
                                nc.vector.scalar_tensor_tensor(hrow[:], t3[:], 0.01, t3[:], ALU.mult, ALU.max)
                                ps2 = pt.tile([128, H], F32, tag="trf")
                                nc.tensor.transpose(ps2[:], hrow[:], ident[:])
                                h2T = wp.tile([128, H], F32, tag="h2T")
                                nc.vector.tensor_copy(h2T[:], ps2[:])
                                for pk, wk in (("p1", "W1aT"), ("p2", "W1bT")):
                                    psp = pt.tile([128, H], F32, tag="pp" + pk)
                                    nc.tensor.matmul(psp[:], h2T[:], consts[wk][:], start=True, stop=True)
                                    pr = wp.tile([128, H], F32, tag="pr" + pk)
                                    nc.vector.tensor_copy(pr[:], psp[:])
                                    nc.sync.dma_start(p_in[pk][t * 128:(t + 1) * 128, :], pr[:])

            h1T = spool.tile([128, NSLOT], BF16, tag="big1")
            gcn_layer(0, h1T)
            if DBG:
                nc.sync.dma_start(dbg["h1T"][:], h1T[:])
            with tc.tile_pool(name="pw", bufs=2) as wp, \
                 tc.tile_pool(name="pp1", bufs=4, space="PSUM") as pp:
                for r in ("sim", "user"):
                    for k in range(NTILE):
                        sl_ = slice(k * 128, (k + 1) * 128)
                        ps = pp.tile([128, H], F32, tag="proj")
                        nc.tensor.matmul(ps[:], h1T[:, sl_], consts[f"W1{r}"][:], start=True, stop=True)
                        ot = wp.tile([128, H], BF16, tag="po")
                        nc.vector.tensor_scalar_mul(ot[:], ps[:], norm[f"odeg_{r}"][:, k:k + 1])
                        nc.sync.dma_start(feat_in["1" + r][sl_, :], ot[:])
                    nc.gpsimd.collective_compute("AllGather", ALU.bypass, replica_groups=RG,
                                                 ins=[feat_in["1" + r][:]], outs=[feat_pair["1" + r][:]])
            gcn_layer(1, None)
            with tc.tile_pool(name="zw", bufs=1) as wp:
                zr = wp.tile([128, H], F32, tag="zr")
                nc.vector.memset(zr[:], 0.0)
                for pk in ("p1", "p2"):
                    nc.sync.dma_start(p_pair[pk][PAIRZ:PAIRZ + 64, :], zr[:])
                    nc.gpsimd.collective_compute("AllGather", ALU.bypass, replica_groups=RG,
                                                 ins=[p_in[pk][:]], outs=[p_pair[pk][0:PAIRZ, :]])
                if DBG:
                    nc.sync.dma_start(dbg["p1"][:], p_pair["p1"][:])
                    nc.sync.dma_start(dbg["p2"][:], p_pair["p2"][:])
                    nc.sync.dma_start(dbg["f0sim"][:], feat_pair["0sim"][:])

            # ---------- classifier ----------
            # z = p1[s] + p2[d] (edge-major, bias-free: BatchNorm cancels cls_b1);
            # batch stats via ones-matmul accumulation + AllReduce; then
            # transpose + fused affine/relu + W2.
            with tc.tile_pool(name="cw", bufs=3) as wp, \
                 tc.tile_pool(name="cz", bufs=1) as zp, \
                 tc.tile_pool(name="cg", bufs=2) as gp, \
                 tc.tile_pool(name="cstat", bufs=1, space="PSUM") as sp, \
                 tc.tile_pool(name="cp", bufs=2, space="PSUM") as pp:
                outst = zp.tile([128, CC * 2], F32, tag="outst")
                sums = zp.tile([128, CC], F32, tag="sums")
                sqs = zp.tile([128, CC], F32, tag="sqs")

                def cls_gather(koff, nb, bi, qs, qd):
                    gs = gp.tile([128, nb, H], F32, tag="cgs")
                    gd = gp.tile([128, nb, H], F32, tag="cgd")
                    s_base = p_pair["p1"][:, 0:H] if bi < 2 else p_pair["p1"][:, H:2 * H]
                    d_base = p_pair["p2"][:, 0:H] if bi % 2 == 0 else p_pair["p2"][:, H:2 * H]
                    nc.gpsimd.dma_gather(gs[:], s_base, idxt["cs_idx"][:, koff * 8:(koff + nb) * 8],
                                         nb * 128, nb * 128, H, elem_step=2 * H,
                                         single_packet=False, queue_num=qs)
                    nc.gpsimd.dma_gather(gd[:], d_base, idxt["cd_idx"][:, koff * 8:(koff + nb) * 8],
                                         nb * 128, nb * 128, H, elem_step=2 * H,
                                         single_packet=False, queue_num=qd)
                    return gs, gd

                koff = 0
                for bi in range(4):
                    nb = meta["CBK"][bi]
                    gs, gd = cls_gather(koff, nb, bi, bi, (bi + 2) % 4)
                    for j in range(nb):
                        k = koff + j
                        wz = wp.tile([128, 128], F32, tag="wz")
                        nc.vector.tensor_add(wz[:], gs[:, j, :], gd[:, j, :])
                        pst = pp.tile([128, 128], F32, tag="ctr")
                        nc.tensor.transpose(pst[:], wz[:], ident[:])
                        zt = wp.tile([128, 128], F32, tag="zt")
                        nc.scalar.activation(zt[:], pst[:], A.Copy, accum_out=sums[:, k:k + 1])
                        sq = wp.tile([128, 128], F32, tag="csq")
                        nc.scalar.activation(sq[:], zt[:], A.Square, accum_out=sqs[:, k:k + 1])
                        nc.sync.dma_start(zt_dram[:, k * 128:(k + 1) * 128], zt[:])
                    koff += nb
                s1 = wp.tile([128, 1], F32, tag="s1")
                nc.vector.tensor_reduce(s1[:], sums[:], mybir.AxisListType.X, ALU.add)
                s2 = wp.tile([128, 1], F32, tag="s2")
                nc.vector.tensor_reduce(s2[:], sqs[:], mybir.AxisListType.X, ALU.add)
                stt = wp.tile([128, 2], F32, tag="stt")
                nc.vector.tensor_copy(stt[:, 0:1], s1[:])
                nc.vector.tensor_copy(stt[:, 1:2], s2[:])
                nc.sync.dma_start(st_in[:], stt[:])
                nc.gpsimd.collective_compute("AllReduce", ALU.add, replica_groups=RG,
                                             ins=[st_in[:]], outs=[st_out[:]])
                stg = wp.tile([128, 2], F32, tag="stg")
                nc.sync.dma_start(stg[:], st_out[:])
                if DBG:
                    nc.sync.dma_start(dbg["st"][:], stg[:])
                stc = wp.tile([128, 2], F32, tag="stc")
                nc.vector.tensor_scalar_mul(stc[:], stg[:], 1.0 / ESUB)
                var = wp.tile([128, 1], F32, tag="var")
                nc.vector.tensor_mul(var[:], stc[:, 0:1], stc[:, 0:1])
                nc.vector.tensor_tensor(out=var[:], in0=stc[:, 1:2], in1=var[:], op=ALU.subtract)
                epsc = wp.tile([128, 1], F32, tag="eps")
                nc.vector.memset(epsc[:], 1e-5)
                sd_ = wp.tile([128, 1], F32, tag="sd")
                nc.scalar.activation(sd_[:], var[:], A.Sqrt, bias=epsc[:, :])
                inv = wp.tile([128, 1], F32, tag="inv")
                nc.vector.reciprocal(inv[:], sd_[:])
                Acol = wp.tile([128, 1], F32, tag="Ac")
                nc.vector.tensor_mul(Acol[:], consts["bngb"][:, 0:1], inv[:])
                Ccol = wp.tile([128, 1], F32, tag="Cc")
                nc.vector.tensor_mul(Ccol[:], stc[:, 0:1], Acol[:])
                nc.vector.tensor_tensor(out=Ccol[:], in0=consts["bngb"][:, 1:2], in1=Ccol[:], op=ALU.subtract)
                for k in range(CC):
                    zt = wp.tile([128, 128], F32, tag="zt2")
                    nc.sync.dma_start(zt[:], zt_dram[:, k * 128:(k + 1) * 128])
                    zn = wp.tile([128, 128], BF16, tag="zn")
                    nc.scalar.activation(zn[:], zt[:], A.Relu, bias=Ccol[:, :], scale=Acol[:, :])
                    pso = pp.tile([128, 2], F32, tag="co")
                    nc.tensor.matmul(pso[:], zn[:], consts["W2T"][:], start=True, stop=True)
                    nc.vector.tensor_add(outst[:, 2 * k:2 * k + 2], pso[:], consts["b2bias"][:])
                nc.sync.dma_start(out[:], outst[:])
    nc.compile()
    return nc


_CACHE = {}


def kernel(**inputs):
    in_maps, out_rows, meta = host_prep(inputs)
    key = (meta["TOTS"], meta["CC"], tuple(meta["gcn"]["sim"]["KE"]),
           tuple(meta["gcn"]["sim"]["KO"]), tuple(meta["gcn"]["user"]["KE"]),
           tuple(meta["gcn"]["user"]["KO"]), tuple(meta["CBK"]))
    if key not in _CACHE:
        _CACHE[key] = build_program(meta)
    nc = _CACHE[key]
    import os
    trace = bool(os.environ.get("KTRACE"))
    res = run_bass_kernel_spmd(nc, in_maps, list(range(NC)), trace=trace)
    if trace and res.exec_time_ns is not None:
        print(f"HW exec time: {res.exec_time_ns} ns")
    outp = np.zeros((ESUB, 2), np.float32)
    CC = meta["CC"]
    for c in range(NC):
        o = np.asarray(res.results[c]["out"], np.float32)
        o = o.reshape(128, CC, 2).transpose(1, 0, 2).reshape(CC * 128, 2)
        rows = out_rows[c]
        m = rows >= 0
        outp[rows[m]] = o[m]
    return outp

